# revision 44
# baseline (speedup 1.0000x reference)
"""InnerProductDecoder Trainium2 kernel.

adj = sigmoid(Zh @ Zh.T) per graph, Zh = Z @ W.T + b,
G=64 graphs x N=1024 nodes, D_IN=256, D_H=128.

Sharding: data-parallel over graphs, 8 graphs per NeuronCore on 8 cores.
W/b replicated. No collectives.  ~59 us HW exec (baseline 120 us).

The kernel is jointly PE/ScalarE-bound: the activity throttle caps the
tensor engine at ~1.2 GHz effective (0.83 ns/row; 6656 matmul rows/graph
= fc1 2048 + triangular S 4608), and ScalarE's sigmoid runs 1 col/cycle
@1.2 GHz over the 4608 upper-triangle columns per graph (~4.6 us/graph
each).  Everything else is arranged to stay off that critical path:

  - Z is transposed + cast to bf16 on the HOST: no PE transposes, half
    the load bytes.  W is host-prepacked into the [p, c, h] stationary
    layout so it loads in one 512B-row DMA.
  - adj is symmetric per graph: only upper-triangular 128-block tiles
    are computed/stored; the host mirrors the rest.  Row tiles are
    grouped into 5 PSUM tiles per graph - t0 (1024), (t1,t7) (896+128),
    (t2,t6) (768+256), (t3,t5) (640+384), t4 (512) - so ScalarE does 5
    sigmoids/graph (fewer instruction overheads) and no matmul segment
    crosses a PSUM bank boundary.
  - PSUM: the four 1024-wide groups rotate through a 3-deep 2-bank pool
    (6 banks); fc1's 512-chunk accumulators and the t4 group share a
    2-deep 1-bank pool (2 banks) = all 8 banks, max pipeline depth.
  - Each graph's sigmoids collect in one SBUF tile [128, 4608] and
    leave in ONE 1.18 MB store to a packed scratch layout (host
    unpacks): 8 store dispatches instead of 64 cuts HWDGE descriptor
    gen from ~38 us to ~5 us.  Stores alternate between the sync and
    gpsimd rings (one ring alone sustains only ~236 GB/s).  The last
    graph stores per-tile, smallest tile last, to trim the drain tail.
  - Z loads: graph 0 in quarters + graphs 1-2 on the sync ring in fc1
    consumption order (j=1 first, matching the fc1 loop); graphs 3-7 on
    the gpsimd ring gated behind graph 1's arrival so their bulk drain
    cannot starve the sync ring during the critical first ~14 us.  wt/b
    ride the otherwise-empty scalar ring.
  - A dummy sigmoid on a memset const right after the wt/b dispatches
    pulls the ~1.3 us ACT_TABLE_LOAD off the critical path.

Numerics: bf16 Z/W/zh + f32 PSUM accumulation + bf16 output gives L2
rel err ~5.1e-3 (gate is 2e-2).  fp8 DoubleRow fc1 (A+R residual split)
was tried and REJECTED: hardware accumulates fp8 products at reduced
precision (L2 1.4e-2) and it ran slower.
"""

import numpy as np

N_CORES = 8
G_PER_CORE = 8
N = 1024          # nodes per graph
D = 256           # input dim
H = 128           # hidden dim
NT = N // 128     # 128-row tiles per graph

# 5 psum groups per graph: (tiles, width); t4 first - it only needs the
# j=1 half of zh, so its matmuls+sigmoid start right after the first
# bias-add (fc1 runs j=1 before j=0). Offsets into the packed output.
GROUPS = [
    ((4,), 512),
    ((0,), 1024),
    ((1, 7), 1024),
    ((2, 6), 1024),
    ((3, 5), 1024),
]
GOFF = [0, 512, 1536, 2560, 3584]   # packed output offsets
PACKW = 4608

_CACHE = {}


def _group_segments(tiles):
    """(psum_off, width, stat_tile, moving_col) segments, chunked at the
    absolute 512-boundaries of the psum tile (bank alignment)."""
    segs = []
    off = 0
    for t in tiles:
        w = (N - 128 * t) if t == tiles[0] else (128 + 128 * (7 - t))
        base = 128 * t
        done = 0
        while done < w:
            nxt = min(w - done, 512 - (off % 512) if off % 512 else 512)
            segs.append((off, nxt, t, base + done))
            off += nxt
            done += nxt
    return segs


def _build_nc():
    import concourse.bass as bass
    import concourse.tile as tile
    from concourse import bacc, mybir
    from concourse._compat import get_trn_type

    f32 = mybir.dt.float32
    bf16 = mybir.dt.bfloat16

    nc = bacc.Bacc(get_trn_type() or "TRN2", target_bir_lowering=False, debug=False)
    ZT_d = nc.declare_dram_parameter("ZT", [D, G_PER_CORE * N], bf16, isOutput=False)
    WT_d = nc.declare_dram_parameter("WT", [128, 2 * H], bf16, isOutput=False)
    b_d = nc.declare_dram_parameter("b", [H, 1], f32, isOutput=False)
    # packed output rows: graph-major, 4608 cols; host unpacks
    adj_d = nc.declare_dram_parameter(
        "adj", [G_PER_CORE * 128, PACKW], bf16, isOutput=True
    )

    with tile.TileContext(nc) as tc:
        with (
            tc.tile_pool(name="consts", bufs=1) as consts,
            tc.tile_pool(name="zt", bufs=G_PER_CORE) as zt_pool,
            tc.tile_pool(name="zh", bufs=3) as zh_pool,
            tc.tile_pool(name="outp", bufs=3) as out_pool,
            tc.tile_pool(name="psf", bufs=2, space=bass.MemorySpace.PSUM) as psf_pool,
            tc.tile_pool(name="pss", bufs=3, space=bass.MemorySpace.PSUM) as pss_pool,
        ):
            # WT_d is host-prepacked as [128, 2*128] with
            # WT_d[p, c*128+h] = W[h, c*128+p]: loads in ONE 512B-row DMA
            WTv = WT_d.rearrange("p (c h) -> p c h", c=2)
            wt = consts.tile([128, 2, H], bf16)
            b_sb = consts.tile([128, 1], f32)
            # ZTv[g, c, p, n] = Z_g[n, c*128 + p]
            ZTv = ZT_d.rearrange("(c p) (g n) -> g c p n", c=2, p=128, g=G_PER_CORE)
            zts = {}
            for g in range(G_PER_CORE):
                zts[g] = zt_pool.tile([128, 2, N], bf16, name="zt")

            # sync ring: dispatch order = fc1 consumption order; each
            # dispatch is ~680 ns serial on the queue, so graph 0's
            # quarters + b + wt go absolutely first, then graphs 1-2.
            # Graphs 3-7 go on the gpsimd (SWDGE) ring, but gated behind a
            # read of zts[1] so their bulk drain cannot starve the sync
            # ring's SDMA share during the critical first ~14 us.
            # tiny wt/b on the (empty) scalar ring: they land ~8.3 us even
            # while the sync ring drains g0's chunks
            nc.scalar.dma_start(wt[:], WTv[:])
            nc.scalar.dma_start(b_sb[:], b_d[:])
            q = lambda j: slice(j * 512, (j + 1) * 512)
            # graph 1 rides the scalar ring in parallel with g0's quarters
            # on sync, so fc1(g1) never waits behind g0's trickle
            for c in range(2):
                nc.scalar.dma_start(zts[1][:, c, :], ZTv[1, c])
            # dummy sigmoid on a memset const (no DMA dep): hoists the
            # ~1.3 us ACT_TABLE_LOAD off the critical path (queued on the
            # scalar ring after the wt/b dispatches, before any real ACT)
            scratch = consts.tile([128, 1], f32)
            nc.vector.memset(scratch[:], 0.0)
            warm = consts.tile([128, 1], f32)
            nc.scalar.activation(
                warm[:], scratch[:], mybir.ActivationFunctionType.Sigmoid
            )
            # g0's quarters all on the sync ring, in fc1 consumption order
            # (j=1 first)
            for j in (1, 0):
                for c in range(2):
                    nc.sync.dma_start(zts[0][:, c, q(j)], ZTv[0, c, :, q(j)])
            for c in range(2):
                nc.sync.dma_start(zts[2][:, c, :], ZTv[2, c])
            gate = consts.tile([128, 1], bf16)
            nc.gpsimd.tensor_copy(gate[:], zts[1][:, 0, 0:1])
            for g in range(3, G_PER_CORE):
                for c in range(2):
                    nc.gpsimd.dma_start(zts[g][:, c, :], ZTv[g, c])

            Pv = adj_d.rearrange("(g p) w -> g p w", g=G_PER_CORE, p=128)

            for g in range(G_PER_CORE):
                zt = zts.pop(g)

                # fc1: Zh^T[h, n] = W @ Z_g^T + b in dedicated 1-bank psum
                # chunks; DVE evicts each (bias add) into bf16 zh.
                zh = zh_pool.tile([128, N], bf16)
                for j in (1, 0):
                    sl = slice(j * 512, (j + 1) * 512)
                    pf = psf_pool.tile([128, 512], f32, name="pf")
                    for c in range(2):
                        nc.tensor.matmul(
                            pf[:], wt[:, c, :], zt[:, c, sl],
                            start=(c == 0), stop=(c == 1),
                        )
                    nc.vector.tensor_scalar_add(zh[:, sl], pf[:], b_sb[:])

                # S groups: t4 (512 wide) shares the 1-bank psf slots, the
                # four 1024-wide groups rotate through the 3-deep 2-bank
                # pool. For the last graph t4 goes LAST so the final store
                # is the smallest (shorter drain tail).
                order = list(range(5))
                if g == G_PER_CORE - 1:
                    order = order[1:] + order[:1]
                ot = out_pool.tile([128, PACKW], bf16)
                for gi in order:
                    tiles, gw = GROUPS[gi]
                    if gw == 512:
                        ps = psf_pool.tile([128, 512], f32, name="pf")
                    else:
                        ps = pss_pool.tile([128, N], f32, name="ps")
                    for off, cw, t, mcol in _group_segments(tiles):
                        nc.tensor.matmul(
                            ps[:, off:off + cw],
                            zh[:, 128 * t:128 * (t + 1)],
                            zh[:, mcol:mcol + cw],
                        )
                    osl = slice(GOFF[gi], GOFF[gi] + gw)
                    nc.scalar.activation(
                        ot[:, osl], ps[:, :gw],
                        mybir.ActivationFunctionType.Sigmoid,
                    )
                    if g == G_PER_CORE - 1:
                        # last graph: store per tile to trim the drain tail
                        nc.sync.dma_start(Pv[g][:, osl], ot[:, osl])
                # alternate store rings: one HWDGE ring alone sustains only
                # ~236 GB/s, below the 9.4 MB store stream's needs.  Even
                # graphs (incl. g6) go on gpsimd so g7's per-tile tail
                # stores never queue behind g6's 1.18 MB on sync.
                if g != G_PER_CORE - 1:
                    eng = nc.gpsimd if (g % 2 == 0 and g > 0) else nc.sync
                    eng.dma_start(Pv[g], ot[:])

    nc.compile()
    return nc


def _get_nc():
    if "nc" not in _CACHE:
        _CACHE["nc"] = _build_nc()
    return _CACHE["nc"]


def run(Z, W, b, trace=False):
    import ml_dtypes
    from concourse.bass_utils import run_bass_kernel_spmd

    bf16 = ml_dtypes.bfloat16
    Z = np.asarray(Z, dtype=np.float32)
    W = np.asarray(W, dtype=np.float32)
    b = np.ascontiguousarray(np.asarray(b, dtype=np.float32)).reshape(H, 1)
    assert Z.shape == (N_CORES * G_PER_CORE * N, D)

    rows = G_PER_CORE * N
    ZT = np.ascontiguousarray(
        Z.reshape(N_CORES, rows, D).transpose(0, 2, 1)
    ).astype(bf16)
    # prepacked so the device sees [p, c*128+h] = W[h, c*128+p]
    WT = np.ascontiguousarray(
        W.reshape(H, 2, 128).transpose(2, 1, 0).reshape(128, 2 * H)
    ).astype(bf16)

    nc = _get_nc()
    in_maps = [{"ZT": ZT[c], "WT": WT, "b": b} for c in range(N_CORES)]
    res = run_bass_kernel_spmd(nc, in_maps, list(range(N_CORES)), trace=trace)

    out = np.empty((N_CORES * G_PER_CORE, N, N), np.float32)
    for c in range(N_CORES):
        R = np.asarray(res.results[c]["adj"]).astype(np.float32)
        R = R.reshape(G_PER_CORE, 128, PACKW)
        oc = out[c * G_PER_CORE:(c + 1) * G_PER_CORE]
        for gi, (tiles, gw) in enumerate(GROUPS):
            off = GOFF[gi]
            for t in tiles:
                w = (N - 128 * t) if t == tiles[0] else (128 + 128 * (7 - t))
                rb = 128 * t
                oc[:, rb:rb + 128, N - w:] = R[:, :, off:off + w]
                off += w
    # mirror the upper-triangle block tiles into the lower triangle
    ob = out.reshape(N_CORES * G_PER_CORE, NT, 128, NT, 128)
    for i in range(NT):
        for j in range(i):
            ob[:, i, :, j, :] = ob[:, j, :, i, :].transpose(0, 2, 1)
    return out, res


def kernel(Z=None, W=None, b=None, node_slice=None, **kwargs):
    out, _ = run(Z, W, b)
    return out


# revision 45
# speedup vs baseline: 1.1476x; 1.1476x over previous
"""InnerProductDecoder Trainium2 kernel.

adj = sigmoid(Zh @ Zh.T) per graph, Zh = Z @ W.T + b,
G=64 graphs x N=1024 nodes, D_IN=256, D_H=128.

Sharding: data-parallel over graphs, 8 graphs per NeuronCore on 8 cores.
W/b replicated. No collectives.  ~59 us HW exec (baseline 120 us).

The kernel is jointly PE/ScalarE-bound: the activity throttle caps the
tensor engine at ~1.2 GHz effective (0.83 ns/row; 6656 matmul rows/graph
= fc1 2048 + triangular S 4608), and ScalarE's sigmoid runs 1 col/cycle
@1.2 GHz over the 4608 upper-triangle columns per graph (~4.6 us/graph
each).  Everything else is arranged to stay off that critical path:

  - Z is transposed + cast to bf16 on the HOST: no PE transposes, half
    the load bytes.  W is host-prepacked into the [p, c, h] stationary
    layout so it loads in one 512B-row DMA.
  - adj is symmetric per graph: only upper-triangular 128-block tiles
    are computed/stored; the host mirrors the rest.  Row tiles are
    grouped into 5 PSUM tiles per graph - t0 (1024), (t1,t7) (896+128),
    (t2,t6) (768+256), (t3,t5) (640+384), t4 (512) - so ScalarE does 5
    sigmoids/graph (fewer instruction overheads) and no matmul segment
    crosses a PSUM bank boundary.
  - PSUM: the four 1024-wide groups rotate through a 3-deep 2-bank pool
    (6 banks); fc1's 512-chunk accumulators and the t4 group share a
    2-deep 1-bank pool (2 banks) = all 8 banks, max pipeline depth.
  - Each graph's sigmoids collect in one SBUF tile [128, 4608] and
    leave in ONE 1.18 MB store to a packed scratch layout (host
    unpacks): 8 store dispatches instead of 64 cuts HWDGE descriptor
    gen from ~38 us to ~5 us.  Stores alternate between the sync and
    gpsimd rings (one ring alone sustains only ~236 GB/s).  The last
    graph stores per-tile, smallest tile last, to trim the drain tail.
  - Z loads: graph 0 in quarters + graphs 1-2 on the sync ring in fc1
    consumption order (j=1 first, matching the fc1 loop); graphs 3-7 on
    the gpsimd ring gated behind graph 1's arrival so their bulk drain
    cannot starve the sync ring during the critical first ~14 us.  wt/b
    ride the otherwise-empty scalar ring.
  - A dummy sigmoid on a memset const right after the wt/b dispatches
    pulls the ~1.3 us ACT_TABLE_LOAD off the critical path.

Numerics: bf16 Z/W/zh + f32 PSUM accumulation + bf16 output gives L2
rel err ~5.1e-3 (gate is 2e-2).  fp8 DoubleRow fc1 (A+R residual split)
was tried and REJECTED: hardware accumulates fp8 products at reduced
precision (L2 1.4e-2) and it ran slower.
"""

import numpy as np

N_CORES = 8
G_PER_CORE = 8
N = 1024          # nodes per graph
D = 256           # input dim
H = 128           # hidden dim
NT = N // 128     # 128-row tiles per graph

# 5 psum groups per graph: (tiles, width); t4 first - it only needs the
# j=1 half of zh, so its matmuls+sigmoid start right after the first
# bias-add (fc1 runs j=1 before j=0). Offsets into the packed output.
GROUPS = [
    ((4,), 512),
    ((0,), 1024),
    ((1, 7), 1024),
    ((2, 6), 1024),
    ((3, 5), 1024),
]
GOFF = [0, 512, 1536, 2560, 3584]   # packed output offsets
PACKW = 4608

_CACHE = {}


def _group_segments(tiles):
    """(psum_off, width, stat_tile, moving_col) segments, chunked at the
    absolute 512-boundaries of the psum tile (bank alignment)."""
    segs = []
    off = 0
    for t in tiles:
        w = (N - 128 * t) if t == tiles[0] else (128 + 128 * (7 - t))
        base = 128 * t
        done = 0
        while done < w:
            nxt = min(w - done, 512 - (off % 512) if off % 512 else 512)
            segs.append((off, nxt, t, base + done))
            off += nxt
            done += nxt
    return segs


def _build_nc():
    import concourse.bass as bass
    import concourse.tile as tile
    from concourse import bacc, mybir
    from concourse._compat import get_trn_type

    f32 = mybir.dt.float32
    bf16 = mybir.dt.bfloat16

    nc = bacc.Bacc(get_trn_type() or "TRN2", target_bir_lowering=False, debug=False)
    ZT_d = nc.declare_dram_parameter("ZT", [D, G_PER_CORE * N], bf16, isOutput=False)
    WT_d = nc.declare_dram_parameter("WT", [128, 2 * H], bf16, isOutput=False)
    b_d = nc.declare_dram_parameter("b", [H, 1], f32, isOutput=False)
    # packed output rows: graph-major, 4608 cols; host unpacks
    adj_d = nc.declare_dram_parameter(
        "adj", [G_PER_CORE * 128, PACKW], bf16, isOutput=True
    )

    with tile.TileContext(nc) as tc:
        with (
            tc.tile_pool(name="consts", bufs=1) as consts,
            tc.tile_pool(name="zt", bufs=G_PER_CORE) as zt_pool,
            tc.tile_pool(name="zh", bufs=3) as zh_pool,
            tc.tile_pool(name="outp", bufs=3) as out_pool,
            tc.tile_pool(name="psf", bufs=2, space=bass.MemorySpace.PSUM) as psf_pool,
            tc.tile_pool(name="pss", bufs=3, space=bass.MemorySpace.PSUM) as pss_pool,
        ):
            # WT_d is host-prepacked as [128, 2*128] with
            # WT_d[p, c*128+h] = W[h, c*128+p]: loads in ONE 512B-row DMA
            WTv = WT_d.rearrange("p (c h) -> p c h", c=2)
            wt = consts.tile([128, 2, H], bf16)
            b_sb = consts.tile([128, 1], f32)
            # ZTv[g, c, p, n] = Z_g[n, c*128 + p]
            ZTv = ZT_d.rearrange("(c p) (g n) -> g c p n", c=2, p=128, g=G_PER_CORE)
            zts = {}
            for g in range(G_PER_CORE):
                zts[g] = zt_pool.tile([128, 2, N], bf16, name="zt")

            # sync ring: dispatch order = fc1 consumption order; each
            # dispatch is ~680 ns serial on the queue, so graph 0's
            # quarters + b + wt go absolutely first, then graphs 1-2.
            # Graphs 3-7 go on the gpsimd (SWDGE) ring, but gated behind a
            # read of zts[1] so their bulk drain cannot starve the sync
            # ring's SDMA share during the critical first ~14 us.
            # tiny wt/b on the (empty) scalar ring: they land ~8.3 us even
            # while the sync ring drains g0's chunks
            nc.scalar.dma_start(wt[:], WTv[:])
            nc.scalar.dma_start(b_sb[:], b_d[:])
            q = lambda j: slice(j * 512, (j + 1) * 512)
            # dummy sigmoid on a memset const (no DMA dep): hoists the
            # ~1.3 us ACT_TABLE_LOAD off the critical path (queued on the
            # scalar ring after the wt/b dispatches, before any real ACT)
            scratch = consts.tile([128, 1], f32)
            nc.vector.memset(scratch[:], 0.0)
            warm = consts.tile([128, 1], f32)
            nc.scalar.activation(
                warm[:], scratch[:], mybir.ActivationFunctionType.Sigmoid
            )
            # g0's quarters all on the sync ring, in fc1 consumption order
            # (j=1 first)
            for j in (1, 0):
                for c in range(2):
                    nc.sync.dma_start(zts[0][:, c, q(j)], ZTv[0, c, :, q(j)])
            for g in range(1, 3):
                for c in range(2):
                    nc.sync.dma_start(zts[g][:, c, :], ZTv[g, c])
            gate = consts.tile([128, 1], bf16)
            nc.gpsimd.tensor_copy(gate[:], zts[1][:, 0, 0:1])
            for g in range(3, G_PER_CORE):
                for c in range(2):
                    nc.gpsimd.dma_start(zts[g][:, c, :], ZTv[g, c])

            Pv = adj_d.rearrange("(g p) w -> g p w", g=G_PER_CORE, p=128)

            for g in range(G_PER_CORE):
                zt = zts.pop(g)

                # fc1: Zh^T[h, n] = W @ Z_g^T + b in dedicated 1-bank psum
                # chunks; DVE evicts each (bias add) into bf16 zh.
                zh = zh_pool.tile([128, N], bf16)
                for j in (1, 0):
                    sl = slice(j * 512, (j + 1) * 512)
                    pf = psf_pool.tile([128, 512], f32, name="pf")
                    for c in range(2):
                        nc.tensor.matmul(
                            pf[:], wt[:, c, :], zt[:, c, sl],
                            start=(c == 0), stop=(c == 1),
                        )
                    nc.vector.tensor_scalar_add(zh[:, sl], pf[:], b_sb[:])

                # S groups: t4 (512 wide) shares the 1-bank psf slots, the
                # four 1024-wide groups rotate through the 3-deep 2-bank
                # pool. For the last graph t4 goes LAST so the final store
                # is the smallest (shorter drain tail).
                order = list(range(5))
                if g == G_PER_CORE - 1:
                    order = order[1:] + order[:1]
                ot = out_pool.tile([128, PACKW], bf16)
                for gi in order:
                    tiles, gw = GROUPS[gi]
                    if gw == 512:
                        ps = psf_pool.tile([128, 512], f32, name="pf")
                    else:
                        ps = pss_pool.tile([128, N], f32, name="ps")
                    for off, cw, t, mcol in _group_segments(tiles):
                        nc.tensor.matmul(
                            ps[:, off:off + cw],
                            zh[:, 128 * t:128 * (t + 1)],
                            zh[:, mcol:mcol + cw],
                        )
                    osl = slice(GOFF[gi], GOFF[gi] + gw)
                    nc.scalar.activation(
                        ot[:, osl], ps[:, :gw],
                        mybir.ActivationFunctionType.Sigmoid,
                    )
                    if g == G_PER_CORE - 1:
                        # last graph: store per tile to trim the drain tail
                        nc.sync.dma_start(Pv[g][:, osl], ot[:, osl])
                # alternate store rings: one HWDGE ring alone sustains only
                # ~236 GB/s, below the 9.4 MB store stream's needs
                if g != G_PER_CORE - 1:
                    # g6 on gpsimd so g7's per-tile tail stores never queue
                    # behind g6's 1.18 MB on the sync ring
                    eng = nc.gpsimd if (g % 2 == 0 and g > 0) else nc.sync
                    eng.dma_start(Pv[g], ot[:])

    nc.compile()
    return nc


def _get_nc():
    if "nc" not in _CACHE:
        _CACHE["nc"] = _build_nc()
    return _CACHE["nc"]


def run(Z, W, b, trace=False):
    import ml_dtypes
    from concourse.bass_utils import run_bass_kernel_spmd

    bf16 = ml_dtypes.bfloat16
    Z = np.asarray(Z, dtype=np.float32)
    W = np.asarray(W, dtype=np.float32)
    b = np.ascontiguousarray(np.asarray(b, dtype=np.float32)).reshape(H, 1)
    assert Z.shape == (N_CORES * G_PER_CORE * N, D)

    rows = G_PER_CORE * N
    ZT = np.ascontiguousarray(
        Z.reshape(N_CORES, rows, D).transpose(0, 2, 1)
    ).astype(bf16)
    # prepacked so the device sees [p, c*128+h] = W[h, c*128+p]
    WT = np.ascontiguousarray(
        W.reshape(H, 2, 128).transpose(2, 1, 0).reshape(128, 2 * H)
    ).astype(bf16)

    nc = _get_nc()
    in_maps = [{"ZT": ZT[c], "WT": WT, "b": b} for c in range(N_CORES)]
    res = run_bass_kernel_spmd(nc, in_maps, list(range(N_CORES)), trace=trace)

    out = np.empty((N_CORES * G_PER_CORE, N, N), np.float32)
    for c in range(N_CORES):
        R = np.asarray(res.results[c]["adj"]).astype(np.float32)
        R = R.reshape(G_PER_CORE, 128, PACKW)
        oc = out[c * G_PER_CORE:(c + 1) * G_PER_CORE]
        for gi, (tiles, gw) in enumerate(GROUPS):
            off = GOFF[gi]
            for t in tiles:
                w = (N - 128 * t) if t == tiles[0] else (128 + 128 * (7 - t))
                rb = 128 * t
                oc[:, rb:rb + 128, N - w:] = R[:, :, off:off + w]
                off += w
    # mirror the upper-triangle block tiles into the lower triangle
    ob = out.reshape(N_CORES * G_PER_CORE, NT, 128, NT, 128)
    for i in range(NT):
        for j in range(i):
            ob[:, i, :, j, :] = ob[:, j, :, i, :].transpose(0, 2, 1)
    return out, res


def kernel(Z=None, W=None, b=None, node_slice=None, **kwargs):
    out, _ = run(Z, W, b)
    return out
